# revision 25
# baseline (speedup 1.0000x reference)
"""RNN-T joint network kernel for 8 Trainium2 NeuronCores.

out[b,t,u,c] = (enc[b,t,:] @ W[:, :D].T)[c] + (dec[b,u,:] @ W[:, D:].T)[c]

Sharding: data-parallel over (b, t-half): core i -> b = i//2, t-slab
[(i%2)*128, (i%2+1)*128).  Each core holds the full W, computes its
(128, 64, 1024) output slab (32 MB) and DMAs it out.  The output DMA
(32 MB/core at ~355-400 GB/s) is the roofline; everything else hides
under or ahead of it.

Host-side prep (part of the sharding/layout strategy): W, enc, dec are
passed pre-transposed so the contraction dim D sits on SBUF partitions
with no on-chip transposes.

Per-core dataflow:
  1. PE warm-up matmuls open the HAM clock gate during the input DMAs.
  2. GEMMs -> dec_proj (split from PSUM into exact bf16 hi+lo halves,
     error ~2^-18) and enc_proj (128,1024) fp32 in SBUF.
  3. For each u: two accumulating K=128 bf16 "selector" matmuls
     broadcast dec_proj[u,:] across all 128 partitions into PSUM.  The
     selector weight is column u of a 128x128 identity, free-dim
     broadcast to all 128 output columns (stride-0 AP) - no
     materialized selector tensor.  DVE adds enc_proj; groups of u's
     form contiguous 0.5-4 MB output DMAs (small first for an early
     first byte, large in steady state, small at the end for a short
     flush).
"""

import sys

import numpy as np

for _p in ("/opt/trn_rl_repo",):
    if _p not in sys.path:
        sys.path.insert(0, _p)

B, T, U, D, C = 4, 256, 64, 512, 1024
TSH = T // 2  # t-slab per core
NCORES = 8

_CACHE = {}


def _build_bass():
    import concourse.mybir as mybir
    from concourse import bacc
    from concourse.bass import ds
    from concourse.masks import make_identity
    from concourse.tile import TileContext

    f32 = mybir.dt.float32
    bf16 = mybir.dt.bfloat16
    add = mybir.AluOpType.add

    nc = bacc.Bacc("TRN2", target_bir_lowering=False, debug=False)
    dect_d = nc.declare_dram_parameter("dect", [D, U], f32, isOutput=False)
    wt_d = nc.declare_dram_parameter("wt", [2 * D, C], f32, isOutput=False)
    enct_d = nc.declare_dram_parameter("enct", [D, TSH], f32, isOutput=False)
    o_d = nc.declare_dram_parameter("o", [TSH, U, C], f32, isOutput=True)

    with TileContext(nc) as tc:
        with (
            tc.tile_pool(name="const", bufs=1) as cpool,
            tc.tile_pool(name="outp", bufs=3) as opool,
        ):
            # bf16 128x128 identity; column u (free-dim broadcast to 128
            # columns) is the K=128 selector weight that broadcasts
            # dec_proj[u,:] across all output partitions.
            ident = cpool.tile([128, 128], bf16)
            make_identity(nc, ident[:])

            # ---- loads; dec side first (its chain to the first selector
            # PSUM is longer), enc side right behind ----
            decT = cpool.tile([128, 4, U], f32)  # decT[p,j,u] = dec[u, j*128+p]
            nc.sync.dma_start(
                out=decT[:], in_=dect_d.rearrange("(j p) u -> p j u", p=128)
            )
            # wT[p, dt, c] = W[c, dt*128+p]; dt 0..3 enc half, 4..7 dec half
            wT = cpool.tile([128, 8, 1024], f32)
            wt_r = wt_d.rearrange("(dt p) c -> p dt c", p=128)
            for dt in (4, 5, 6, 7):
                nc.sync.dma_start(out=wT[:, dt, :], in_=wt_r[:, dt, :])
            encT = cpool.tile([128, 4, TSH], f32)
            nc.sync.dma_start(
                out=encT[:], in_=enct_d.rearrange("(j p) t -> p j t", p=128)
            )
            for dt in (0, 1, 2, 3):
                nc.sync.dma_start(out=wT[:, dt, :], in_=wt_r[:, dt, :])

            enc_proj = cpool.tile([TSH, C], f32)
            # dec_proj = dec_hi + dec_lo, both bf16 (exact split to ~2^-18);
            # rows U..127 zero so K=128 matmuls pick up nothing from them.
            dec_hi = cpool.tile([128, C], bf16)
            dec_lo = cpool.tile([128, C], bf16)
            nc.vector.memset(dec_hi[U:, :], 0.0)
            nc.vector.memset(dec_lo[U:, :], 0.0)

            # PE warm-up source tile (zeroed; content irrelevant).
            warm_a = cpool.tile([128, 512], bf16)
            nc.vector.memset(warm_a[:], 0.0)

            with tc.tile_pool(name="psS", bufs=2, space="PSUM") as ppool:
                # PE warm-up: dependency-free matmuls issued while the input
                # DMAs stream, so the HAM clock gate opens (1.2 -> 2.4 GHz)
                # before the projection matmuls run.  Results are discarded.
                # The dummy ScalarE copy pulls the one-time ACT_TABLE_LOAD
                # (~1.3us) off the enc_proj critical path.
                wp = ppool.tile([128, 512], f32, tag="warm")
                for _ in range(10):
                    nc.tensor.matmul(
                        wp[:], warm_a[:, :128], warm_a[:], start=True, stop=True
                    )
                nc.scalar.copy(out=warm_a[:1, :32], in_=wp[:1, :32])

                for h in range(2):
                    pp = ppool.tile([TSH, 512], f32, tag="projd")
                    for dt in range(4):
                        nc.tensor.matmul(
                            pp[:U],
                            decT[:, dt, :],
                            wT[:, 4 + dt, ds(h * 512, 512)],
                            start=(dt == 0),
                            stop=(dt == 3),
                        )
                    # hi/lo split straight from PSUM (no fp32 staging copy):
                    # ACT casts to bf16, DVE computes the bf16 remainder.
                    nc.scalar.copy(out=dec_hi[:U, ds(h * 512, 512)], in_=pp[:U])
                    nc.vector.tensor_tensor(
                        out=dec_lo[:U, ds(h * 512, 512)],
                        in0=pp[:U],
                        in1=dec_hi[:U, ds(h * 512, 512)],
                        op=mybir.AluOpType.subtract,
                    )

                for h in range(2):
                    pp = ppool.tile([TSH, 512], f32, tag="proje")
                    for dt in range(4):
                        nc.tensor.matmul(
                            pp[:],
                            encT[:, dt, :],
                            wT[:, dt, ds(h * 512, 512)],
                            start=(dt == 0),
                            stop=(dt == 3),
                        )
                    nc.scalar.copy(out=enc_proj[:, ds(h * 512, 512)], in_=pp[:])

            # ---- main loop over u ----
            # small at the start (early first output byte), big in the
            # middle (descriptor efficiency), small at the end (short flush)
            groups = [1, 1, 2, 4] + [8] * 5 + [4] * 3 + [2, 1, 1]
            assert sum(groups) == U
            with tc.tile_pool(name="psM", bufs=2, space="PSUM") as mpool:
                u0 = 0
                for gsz in groups:
                    ot = opool.tile([TSH, gsz, C], f32, tag="out")
                    for jp in range((gsz + 1) // 2):
                        uw = min(2, gsz - jp * 2)  # u's in this psum tile
                        pr = mpool.tile([TSH, 2, C], f32, tag="rep")
                        for j2 in range(uw):
                            u = u0 + jp * 2 + j2
                            selw = ident[:, u, None].to_broadcast([128, 128])
                            for h in range(2):
                                nc.tensor.matmul(
                                    pr[:, j2, ds(h * 512, 512)],
                                    selw,
                                    dec_hi[:, ds(h * 512, 512)],
                                    start=True,
                                    stop=False,
                                )
                                nc.tensor.matmul(
                                    pr[:, j2, ds(h * 512, 512)],
                                    selw,
                                    dec_lo[:, ds(h * 512, 512)],
                                    start=False,
                                    stop=True,
                                )
                        nc.vector.tensor_tensor(
                            out=ot[:, ds(jp * 2, uw), :],
                            in0=pr[:, :uw, :],
                            in1=enc_proj[:, None, :].to_broadcast([TSH, uw, C]),
                            op=add,
                        )
                    nc.sync.dma_start(
                        out=o_d[:, ds(u0, gsz), :], in_=ot[:, :gsz, :]
                    )
                    u0 += gsz

    nc.compile()
    return nc


def _get_nc():
    if "nc" not in _CACHE:
        _CACHE["nc"] = _build_bass()
    return _CACHE["nc"]


def _make_in_maps(encoder_outputs, decoder_outputs, W):
    enc = np.asarray(encoder_outputs, dtype=np.float32)
    dec = np.asarray(decoder_outputs, dtype=np.float32)
    w = np.asarray(W, dtype=np.float32)

    wt = np.ascontiguousarray(w.T)  # (2D, C)

    in_maps = []
    for i in range(NCORES):
        b, th = i // 2, i % 2
        enct = np.ascontiguousarray(enc[b, th * TSH : (th + 1) * TSH].T)  # (D, TSH)
        dect = np.ascontiguousarray(dec[b].T)  # (D, U)
        in_maps.append({"enct": enct, "dect": dect, "wt": wt})
    return in_maps


def _run(encoder_outputs, decoder_outputs, W, trace=False):
    from concourse.bass_utils import run_bass_kernel_spmd

    nc = _get_nc()
    in_maps = _make_in_maps(encoder_outputs, decoder_outputs, W)
    res = run_bass_kernel_spmd(nc, in_maps, list(range(NCORES)), trace=trace)
    out = np.empty((B, T, U, C), dtype=np.float32)
    for i in range(NCORES):
        b, th = i // 2, i % 2
        out[b, th * TSH : (th + 1) * TSH] = res.results[i]["o"]
    return out, res


def kernel(encoder_outputs, decoder_outputs, W):
    out, _ = _run(encoder_outputs, decoder_outputs, W)
    return out


# revision 27
# speedup vs baseline: 1.1260x; 1.1260x over previous
"""RNN-T joint network kernel for 8 Trainium2 NeuronCores.

out[b,t,u,c] = (enc[b,t,:] @ W[:, :D].T)[c] + (dec[b,u,:] @ W[:, D:].T)[c]

Sharding: data-parallel over (b, t-half): core i -> b = i//2, t-slab
[(i%2)*128, (i%2+1)*128).  Each core holds the full W, computes its
(128, 64, 1024) output slab (32 MB) and DMAs it out.  The output DMA
(32 MB/core at ~355-400 GB/s) is the roofline; everything else hides
under or ahead of it.

Host-side prep (part of the sharding/layout strategy): W, enc, dec are
passed pre-transposed so the contraction dim D sits on SBUF partitions
with no on-chip transposes.

Per-core dataflow:
  1. PE warm-up matmuls open the HAM clock gate during the input DMAs.
  2. GEMMs -> dec_proj (split from PSUM into exact bf16 hi+lo halves,
     error ~2^-18) and enc_proj (128,1024) fp32 in SBUF.
  3. For each u: two accumulating K=128 bf16 "selector" matmuls
     broadcast dec_proj[u,:] across all 128 partitions into PSUM.  The
     selector weight is column u of a 128x128 identity, free-dim
     broadcast to all 128 output columns (stride-0 AP) - no
     materialized selector tensor.  DVE adds enc_proj; groups of u's
     form contiguous 0.5-4 MB output DMAs (small first for an early
     first byte, large in steady state, small at the end for a short
     flush).
"""

import sys

import numpy as np

for _p in ("/opt/trn_rl_repo",):
    if _p not in sys.path:
        sys.path.insert(0, _p)

B, T, U, D, C = 4, 256, 64, 512, 1024
TSH = T // 2  # t-slab per core
NCORES = 8

_CACHE = {}


def _build_bass():
    import concourse.mybir as mybir
    from concourse import bacc
    from concourse.bass import ds
    from concourse.masks import make_identity
    from concourse.tile import TileContext

    f32 = mybir.dt.float32
    bf16 = mybir.dt.bfloat16
    add = mybir.AluOpType.add

    nc = bacc.Bacc("TRN2", target_bir_lowering=False, debug=False)
    dect_d = nc.declare_dram_parameter("dect", [D, U], f32, isOutput=False)
    wt_d = nc.declare_dram_parameter("wt", [2 * D, C], f32, isOutput=False)
    enct_d = nc.declare_dram_parameter("enct", [D, TSH], f32, isOutput=False)
    o_d = nc.declare_dram_parameter("o", [TSH, U, C], f32, isOutput=True)

    with TileContext(nc) as tc:
        with (
            tc.tile_pool(name="const", bufs=1) as cpool,
            tc.tile_pool(name="outp", bufs=3) as opool,
        ):
            # sel[k, u, m] = 1.0 if k == u else 0.0 (k on partitions; rows
            # U..127 all zero so the selector matmuls are K=128 full-array
            # ops).  Built on the otherwise-idle GpSimd.
            sel = cpool.tile([128, U, 128], bf16)
            nc.gpsimd.memset(sel[:], 0.0)
            nc.gpsimd.affine_select(
                out=sel[:],
                in_=sel[:],
                compare_op=mybir.AluOpType.not_equal,
                fill=1.0,
                base=0,
                pattern=[[-1, U], [0, 128]],
                channel_multiplier=1,
            )

            # ---- loads; dec side first (its chain to the first selector
            # PSUM is longer), enc side right behind ----
            decT = cpool.tile([128, 4, U], f32)  # decT[p,j,u] = dec[u, j*128+p]
            nc.sync.dma_start(
                out=decT[:], in_=dect_d.rearrange("(j p) u -> p j u", p=128)
            )
            # wT[p, dt, c] = W[c, dt*128+p]; dt 0..3 enc half, 4..7 dec half
            wT = cpool.tile([128, 8, 1024], f32)
            wt_r = wt_d.rearrange("(dt p) c -> p dt c", p=128)
            for dt in (4, 5, 6, 7):
                nc.sync.dma_start(out=wT[:, dt, :], in_=wt_r[:, dt, :])
            encT = cpool.tile([128, 4, TSH], f32)
            nc.sync.dma_start(
                out=encT[:], in_=enct_d.rearrange("(j p) t -> p j t", p=128)
            )
            for dt in (0, 1, 2, 3):
                nc.sync.dma_start(out=wT[:, dt, :], in_=wt_r[:, dt, :])

            enc_proj = cpool.tile([TSH, C], f32)
            # dec_proj = dec_hi + dec_lo, both bf16 (exact split to ~2^-18);
            # rows U..127 zero so K=128 matmuls pick up nothing from them.
            dec_hi = cpool.tile([128, C], bf16)
            dec_lo = cpool.tile([128, C], bf16)
            nc.vector.memset(dec_hi[U:, :], 0.0)
            nc.vector.memset(dec_lo[U:, :], 0.0)

            # PE warm-up source tile (zeroed; content irrelevant).
            warm_a = cpool.tile([128, 512], bf16)
            nc.vector.memset(warm_a[:], 0.0)

            with tc.tile_pool(name="psS", bufs=2, space="PSUM") as ppool:
                # PE warm-up: dependency-free matmuls issued while the input
                # DMAs stream, so the HAM clock gate opens (1.2 -> 2.4 GHz)
                # before the projection matmuls run.  Results are discarded.
                # The dummy ScalarE copy pulls the one-time ACT_TABLE_LOAD
                # (~1.3us) off the enc_proj critical path.
                wp = ppool.tile([128, 512], f32, tag="warm")
                for _ in range(10):
                    nc.tensor.matmul(
                        wp[:], warm_a[:, :128], warm_a[:], start=True, stop=True
                    )
                nc.scalar.copy(out=warm_a[:1, :32], in_=wp[:1, :32])

                for h in range(2):
                    pp = ppool.tile([TSH, 512], f32, tag="projd")
                    for dt in range(4):
                        nc.tensor.matmul(
                            pp[:U],
                            decT[:, dt, :],
                            wT[:, 4 + dt, ds(h * 512, 512)],
                            start=(dt == 0),
                            stop=(dt == 3),
                        )
                    # hi/lo split straight from PSUM (no fp32 staging copy):
                    # ACT casts to bf16, DVE computes the bf16 remainder.
                    nc.scalar.copy(out=dec_hi[:U, ds(h * 512, 512)], in_=pp[:U])
                    nc.vector.tensor_tensor(
                        out=dec_lo[:U, ds(h * 512, 512)],
                        in0=pp[:U],
                        in1=dec_hi[:U, ds(h * 512, 512)],
                        op=mybir.AluOpType.subtract,
                    )

                for h in range(2):
                    pp = ppool.tile([TSH, 512], f32, tag="proje")
                    for dt in range(4):
                        nc.tensor.matmul(
                            pp[:],
                            encT[:, dt, :],
                            wT[:, dt, ds(h * 512, 512)],
                            start=(dt == 0),
                            stop=(dt == 3),
                        )
                    nc.scalar.copy(out=enc_proj[:, ds(h * 512, 512)], in_=pp[:])

            # ---- main loop over u ----
            # small at the start (early first output byte), big in the
            # middle (descriptor efficiency), small at the end (short flush)
            groups = [1, 1, 2, 4] + [8] * 5 + [4] * 3 + [2, 1, 1]
            assert sum(groups) == U
            with tc.tile_pool(name="psM", bufs=2, space="PSUM") as mpool:
                u0 = 0
                for gsz in groups:
                    ot = opool.tile([TSH, gsz, C], f32, tag="out")
                    for jp in range((gsz + 1) // 2):
                        uw = min(2, gsz - jp * 2)  # u's in this psum tile
                        pr = mpool.tile([TSH, 2, C], f32, tag="rep")
                        for j2 in range(uw):
                            u = u0 + jp * 2 + j2
                            selw = sel[:, u, :]
                            for h in range(2):
                                nc.tensor.matmul(
                                    pr[:, j2, ds(h * 512, 512)],
                                    selw,
                                    dec_hi[:, ds(h * 512, 512)],
                                    start=True,
                                    stop=False,
                                )
                                nc.tensor.matmul(
                                    pr[:, j2, ds(h * 512, 512)],
                                    selw,
                                    dec_lo[:, ds(h * 512, 512)],
                                    start=False,
                                    stop=True,
                                )
                        nc.vector.tensor_tensor(
                            out=ot[:, ds(jp * 2, uw), :],
                            in0=pr[:, :uw, :],
                            in1=enc_proj[:, None, :].to_broadcast([TSH, uw, C]),
                            op=add,
                        )
                    nc.sync.dma_start(
                        out=o_d[:, ds(u0, gsz), :], in_=ot[:, :gsz, :]
                    )
                    u0 += gsz

    nc.compile()
    return nc


def _get_nc():
    if "nc" not in _CACHE:
        _CACHE["nc"] = _build_bass()
    return _CACHE["nc"]


def _make_in_maps(encoder_outputs, decoder_outputs, W):
    enc = np.asarray(encoder_outputs, dtype=np.float32)
    dec = np.asarray(decoder_outputs, dtype=np.float32)
    w = np.asarray(W, dtype=np.float32)

    wt = np.ascontiguousarray(w.T)  # (2D, C)

    in_maps = []
    for i in range(NCORES):
        b, th = i // 2, i % 2
        enct = np.ascontiguousarray(enc[b, th * TSH : (th + 1) * TSH].T)  # (D, TSH)
        dect = np.ascontiguousarray(dec[b].T)  # (D, U)
        in_maps.append({"enct": enct, "dect": dect, "wt": wt})
    return in_maps


def _run(encoder_outputs, decoder_outputs, W, trace=False):
    from concourse.bass_utils import run_bass_kernel_spmd

    nc = _get_nc()
    in_maps = _make_in_maps(encoder_outputs, decoder_outputs, W)
    res = run_bass_kernel_spmd(nc, in_maps, list(range(NCORES)), trace=trace)
    out = np.empty((B, T, U, C), dtype=np.float32)
    for i in range(NCORES):
        b, th = i // 2, i % 2
        out[b, th * TSH : (th + 1) * TSH] = res.results[i]["o"]
    return out, res


def kernel(encoder_outputs, decoder_outputs, W):
    out, _ = _run(encoder_outputs, decoder_outputs, W)
    return out
